# revision 1
# baseline (speedup 1.0000x reference)
"""Causal self-attention Bass kernel for TRN2, 8 NeuronCores.

Sharding: data-parallel over batch (B=4) x tensor-parallel over head halves
(2 groups of 8 heads) = 8 shards, Megatron-style. Each core computes its
batch's qkv projection for its 8 heads, causal attention, and a partial
output projection (its heads' rows of W_proj). The host sums the two
partials per batch and adds b_proj.

All matmul operands are fp16 (full-rate 1 cycle/row on the PE, fp32 PSUM
accumulation; fp16's 10 mantissa bits keep end-to-end rel err ~4e-4).

Layouts per core:
  xt   = x[b].T (fp16)                 (C=1024, T=2048)
  wqk  = [Wq_half | Wk_half] (fp16)    (1024, 1024)
  wv   = Wv_half (fp16)                (1024, 512)
  wp   = W_proj[512*h2:+512, :] (fp16) (512, 1024)
  QT/KT tiles [128, 512] fp16: partitions = d + 64*(h%2) for head pair h//2
  V tiles [128, 8, 65] fp16: per s-chunk, 8 heads x (64 V cols + ones col)
  scores^T [s,t] (2-head row-packed, K=64, diag cols clipped) -> ACT exp
  -> DVE causal mask-mul -> PV matmul M=65 -> O^T[d,t] + Z row in PSUM
  -> recip_approx_fast + gpsimd partition_broadcast -> normalized OCT (SBUF)
  proj: out[t, c] = sum_hd OCT[hd, t] * wp[hd, c]

Emission interleaves phase-1 QK chunks and output-projection chunks into
the ACT-heavy attention rounds so the PE static order has filler work.
"""

import math
import os

import numpy as np

import concourse.bass as bass
import concourse.mybir as mybir
from concourse import bacc
from concourse.tile import TileContext

F32 = mybir.dt.float32
F32R = mybir.dt.float32r
BF16 = mybir.dt.bfloat16
F16 = mybir.dt.float16

N_EMBD = 1024
N_HEAD = 16
D = 64
B = 4
T = 2048
N_CORES = 8
PAIRS = 4          # head pairs per core (8 heads)
TJ = T // 512      # 512-wide t super-chunks
SJ = T // 128      # 128-wide s chunks
SCALE = 1.0 / math.sqrt(D)

_CACHE = {}


def _build():
    nc = bacc.Bacc()

    xt_d = nc.declare_dram_parameter("xt", [N_EMBD, T], F16, isOutput=False)
    wqk_d = nc.declare_dram_parameter("wqk", [N_EMBD, 1024], F16, isOutput=False)
    wv_d = nc.declare_dram_parameter("wv", [N_EMBD, 512], F16, isOutput=False)
    wp_d = nc.declare_dram_parameter("wp", [512, N_EMBD], F16, isOutput=False)
    bqk_d = nc.declare_dram_parameter("bqk", [128, 8], F32, isOutput=False)
    bv_d = nc.declare_dram_parameter("bv", [1, 512], F16, isOutput=False)
    out_d = nc.declare_dram_parameter("out_p", [T, N_EMBD], F32, isOutput=True)


    with TileContext(nc) as tc:
        with (
            tc.tile_pool(name="const", bufs=1) as cpool,
            tc.tile_pool(name="w", bufs=1) as wpool,
            tc.tile_pool(name="xt", bufs=20) as xpool,
            tc.tile_pool(name="qkt", bufs=1) as qkpool,
            tc.tile_pool(name="v", bufs=1) as vpool,
            tc.tile_pool(name="e", bufs=8) as epool,
            tc.tile_pool(name="octp", bufs=1) as octpool,
            tc.tile_pool(name="misc", bufs=3) as mpool,
            tc.tile_pool(name="outp", bufs=3) as opool,
            tc.tile_pool(name="ps", bufs=2, space="PSUM") as pspool,
            tc.tile_pool(name="pv", bufs=4, space="PSUM") as pvpool,
        ):
            # ---- constants ----
            ones_f = cpool.tile([1, 128], F32, tag="ones_f")
            nc.vector.memset(ones_f, 1.0)
            ones_r = cpool.tile([1, 128], F16, tag="ones_r")
            nc.vector.tensor_copy(ones_r, ones_f)
            ones8 = cpool.tile([128, 8], F32, tag="ones8")
            nc.vector.memset(ones8, 1.0)
            bqk_t = cpool.tile([128, 8], F32, tag="bqk")
            nc.sync.dma_start(out=bqk_t, in_=bqk_d[:, :])
            bv_t = cpool.tile([1, 512], F16, tag="bv")
            nc.sync.dma_start(out=bv_t, in_=bv_d[:, :])

            # bvb = b_v broadcast to [128, 512] via K=1 matmul
            ps_bvb = pvpool.tile([128, 512], F32, tag="pv")
            nc.tensor.matmul(
                ps_bvb, lhsT=ones_r[0:1, :], rhs=bv_t, start=True, stop=True
            )
            bvb = cpool.tile([128, 512], F32, tag="bvb")
            nc.vector.tensor_copy(bvb, ps_bvb)

            # causal masks for the 4 diagonal offsets: keep where f - p - 128k >= 0
            masks = []
            for k in range(4):
                mk = cpool.tile([128, 512], F16, tag=f"mask{k}")
                nc.vector.memset(mk, 1.0)
                nc.gpsimd.affine_select(
                    out=mk, in_=mk, compare_op=mybir.AluOpType.is_ge, fill=0.0,
                    base=-128 * k, pattern=[[1, 512]], channel_multiplier=-1,
                )
                masks.append(mk)

            # ---- weights (wv first: the V pass is the first consumer) ----
            wqk = []
            wv = []
            wp = []
            def load_xt(tj):
                xts = []
                for c in range(8):
                    t = xpool.tile([128, 512], F16, tag="xt")
                    nc.sync.dma_start(
                        out=t,
                        in_=xt_d[128 * c : 128 * c + 128, 512 * tj : 512 * tj + 512],
                    )
                    xts.append(t)
                return xts

            xts0 = []
            for c in range(8):
                t = wpool.tile([128, 512], F16, tag=f"wv{c}")
                nc.sync.dma_start(out=t, in_=wv_d[128 * c : 128 * c + 128, :])
                wv.append(t)
                t2_ = xpool.tile([128, 512], F16, tag="xt")
                nc.sync.dma_start(
                    out=t2_, in_=xt_d[128 * c : 128 * c + 128, 0:512]
                )
                xts0.append(t2_)

            for c in range(8):
                t = wpool.tile([128, 1024], F16, tag=f"wqk{c}")
                nc.sync.dma_start(out=t, in_=wqk_d[128 * c : 128 * c + 128, :])
                wqk.append(t)
            for p in range(PAIRS):
                t = wpool.tile([128, 1024], F16, tag=f"wp{p}")
                nc.sync.dma_start(out=t, in_=wp_d[128 * p : 128 * p + 128, :])
                wp.append(t)

            QT = [[None] * TJ for _ in range(PAIRS)]
            KT = [[None] * TJ for _ in range(PAIRS)]
            V = [None] * SJ
            OCT = [[None] * TJ for _ in range(PAIRS)]

            def v_chunk(tj, sj, xts):
                s_idx = 4 * tj + sj
                pv = pvpool.tile([128, 512], F32, tag="pv")
                for c in range(8):
                    nc.tensor.matmul(
                        pv,
                        lhsT=xts[c][:, 128 * sj : 128 * sj + 128],
                        rhs=wv[c],
                        start=(c == 0),
                        stop=(c == 7),
                    )
                vt = vpool.tile([128, 8, 65], F16, tag=f"v{s_idx}")
                nc.vector.tensor_add(
                    vt[:, :, 0:64],
                    pv.rearrange("p (h d) -> p h d", h=8),
                    bvb.rearrange("p (h d) -> p h d", h=8),
                )
                nc.vector.tensor_copy(
                    vt[:, :, 64:65], ones8.rearrange("p (h o) -> p h o", h=8)
                )
                V[s_idx] = vt

            def qk_chunk(tj, n, xts):
                ps = pspool.tile([128, 1024], F32, tag="ps")
                for c in range(8):
                    nc.tensor.matmul(
                        ps[:, 0:512],
                        lhsT=wqk[c][:, 128 * n : 128 * n + 128],
                        rhs=xts[c],
                        start=(c == 0),
                        stop=(c == 7),
                    )
                dst = qkpool.tile([128, 512], F16, tag=f"qk{n}_{tj}")
                nc.vector.tensor_scalar_add(dst, ps[:, 0:512], bqk_t[:, n : n + 1])
                if n < 4:
                    QT[n][tj] = dst
                else:
                    KT[n - 4][tj] = dst

            def attention_block(pair, tcj):
                nk = 4 * tcj + 4  # kept s-chunks (causal)
                pv1 = pvpool.tile([128, 512], F32, tag="pv")
                pv2 = pvpool.tile([128, 512], F32, tag="pv")
                qt = QT[pair][tcj]
                for si in range(nk):
                    kt = KT[pair][si // 4]
                    koff = 128 * (si % 4)
                    f0 = max(0, 128 * (si - 4 * tcj))  # cols < f0 are fully masked
                    ps = pspool.tile([128, 1024], F32, tag="ps")
                    psv = ps.rearrange("p (g f) -> p g f", g=2)
                    nc.tensor.matmul(
                        psv[:, 0, f0:512],
                        lhsT=kt[0:64, koff : koff + 128],
                        rhs=qt[0:64, f0:512],
                        start=True,
                        stop=True,
                        tile_position=(0, 0),
                    )
                    nc.tensor.matmul(
                        psv[:, 1, f0:512],
                        lhsT=kt[64:128, koff : koff + 128],
                        rhs=qt[64:128, f0:512],
                        start=True,
                        stop=True,
                        tile_position=(64, 0),
                    )
                    et = epool.tile([128, 2, 512], F16, tag="e")
                    nc.scalar.activation(
                        out=et[:, :, f0:512],
                        in_=psv[:, :, f0:512],
                        func=mybir.ActivationFunctionType.Exp,
                        scale=SCALE,
                    )
                    if si >= 4 * tcj:
                        k = si - 4 * tcj
                        nc.vector.tensor_mul(
                            et[:, 0, f0:512], et[:, 0, f0:512], masks[k][:, f0:512]
                        )
                        nc.vector.tensor_mul(
                            et[:, 1, f0:512], et[:, 1, f0:512], masks[k][:, f0:512]
                        )
                    h1 = 2 * pair
                    h2 = 2 * pair + 1
                    nc.tensor.matmul(
                        pv1[0:65, f0:512],
                        lhsT=V[si][:, h1, :],
                        rhs=et[:, 0, f0:512],
                        start=(si == 0),
                        stop=(si == nk - 1),
                    )
                    nc.tensor.matmul(
                        pv2[0:65, f0:512],
                        lhsT=V[si][:, h2, :],
                        rhs=et[:, 1, f0:512],
                        start=(si == 0),
                        stop=(si == nk - 1),
                    )
                # normalize: O[d, t] / Z[t]  (Z in psum row 0)
                oct_t = octpool.tile([128, 512], F16, tag=f"oct{pair}_{tcj}")
                OCT[pair][tcj] = oct_t
                for g, pv in enumerate((pv1, pv2)):
                    rz = mpool.tile([1, 512], F32, tag="rz")
                    nc.vector.tensor_copy(rz, pv[64:65, :])
                    nc.vector.reciprocal_approx_fast(out=rz, in_=rz)
                    # broadcast [1,512] -> [64,512] (gpsimd partition broadcast)
                    rzb = mpool.tile([64, 512], F32, tag="rzb")
                    nc.gpsimd.partition_broadcast(rzb, rz)
                    nc.vector.tensor_mul(
                        oct_t[64 * g : 64 * g + 64, :], pv[0:64, :], rzb
                    )

            def proj_chunk(t2, evict_on_act=False):
                tcj, k = t2 // 4, t2 % 4
                octc = [OCT[pair][tcj][:, 128 * k : 128 * k + 128] for pair in range(PAIRS)]
                for cj in range(2):
                    po = pvpool.tile([128, 512], F32, tag="pv")
                    for pair in range(PAIRS):
                        nc.tensor.matmul(
                            po,
                            lhsT=octc[pair],
                            rhs=wp[pair][:, 512 * cj : 512 * cj + 512],
                            start=(pair == 0),
                            stop=(pair == 3),
                        )
                    ot = opool.tile([128, 512], F32, tag="out")
                    if evict_on_act:
                        # tail chunks: ScalarE is idle once the last exp is done
                        nc.scalar.copy(ot, po)
                    else:
                        nc.vector.tensor_copy(ot, po)
                    nc.sync.dma_start(
                        out=out_d[
                            128 * t2 : 128 * t2 + 128, 512 * cj : 512 * cj + 512
                        ],
                        in_=ot,
                    )

            # ---- pipelined emission: phase1, attention, interleaved proj ----
            # proj for t-range of round tj-1 is striped across round tj's
            # attention blocks so the PE has filler work while ACT runs exp.
            xts_cur = xts0
            for tj in range(TJ):
                for sj in range(4):
                    v_chunk(tj, sj, xts_cur)
                if tj == 0:
                    for p in range(PAIRS):
                        qk_chunk(0, p, xts_cur)
                        qk_chunk(0, 4 + p, xts_cur)
                xts_next = load_xt(tj + 1) if tj + 1 < TJ else None
                for pair in range(PAIRS):
                    attention_block(pair, tj)
                    if tj == 2:
                        proj_chunk(0 + pair)
                    elif tj == 3:
                        proj_chunk(4 + pair)
                        proj_chunk(8 + pair)
                    if tj + 1 < TJ:
                        qk_chunk(tj + 1, pair, xts_next)
                        qk_chunk(tj + 1, 4 + pair, xts_next)
                xts_cur = xts_next
            for t2 in range(12, 16):
                proj_chunk(t2, evict_on_act=True)

    nc.finalize()
    return nc


def _get_nc():
    if "nc" not in _CACHE:
        _CACHE["nc"] = _build()
    return _CACHE["nc"]


def kernel(x, W_qkv, b_qkv, W_proj, b_proj):
    from concourse.bass_utils import run_bass_kernel_spmd

    x = np.asarray(x, dtype=np.float32)
    W_qkv = np.asarray(W_qkv, dtype=np.float32)
    b_qkv = np.asarray(b_qkv, dtype=np.float32)
    W_proj = np.asarray(W_proj, dtype=np.float32)
    b_proj = np.asarray(b_proj, dtype=np.float32)

    in_maps = []
    for core in range(N_CORES):
        b = core // 2
        h2 = core % 2
        o = 512 * h2
        xt = np.ascontiguousarray(x[b].T).astype(np.float16)
        wq = W_qkv[:, o : o + 512]
        wk = W_qkv[:, 1024 + o : 1024 + o + 512]
        wqk = np.ascontiguousarray(np.concatenate([wq, wk], axis=1)).astype(np.float16)
        wv = np.ascontiguousarray(W_qkv[:, 2048 + o : 2048 + o + 512]).astype(np.float16)
        wp = np.ascontiguousarray(W_proj[o : o + 512, :]).astype(np.float16)
        bq = b_qkv[o : o + 512]
        bk = b_qkv[1024 + o : 1024 + o + 512]
        bqk = np.ascontiguousarray(
            np.concatenate([bq, bk]).reshape(8, 128).T
        )
        bv = np.ascontiguousarray(b_qkv[2048 + o : 2048 + o + 512].reshape(1, 512)).astype(np.float16)
        in_maps.append(
            {"xt": xt, "wqk": wqk, "wv": wv, "wp": wp, "bqk": bqk, "bv": bv}
        )

    nc = _get_nc()
    kwargs = {}
    if os.environ.get("BASS_KERNEL_TRACE"):
        kwargs["trace"] = True
    res = run_bass_kernel_spmd(nc, in_maps, core_ids=list(range(N_CORES)), **kwargs)
    _CACHE["last_results"] = res

    out = np.empty((B, T, N_EMBD), dtype=np.float32)
    for b in range(B):
        out[b] = (
            res.results[2 * b]["out_p"]
            + res.results[2 * b + 1]["out_p"]
            + b_proj[None, :]
        )
    return out



# revision 6
# speedup vs baseline: 1.0124x; 1.0124x over previous
"""Causal self-attention Bass kernel for TRN2, 8 NeuronCores.

Sharding: data-parallel over batch (B=4) x tensor-parallel over head halves
(2 groups of 8 heads) = 8 shards, Megatron-style. Each core computes its
batch's qkv projection for its 8 heads, causal attention, and a partial
output projection (its heads' rows of W_proj). The host sums the two
partials per batch and adds b_proj.

All matmul operands are fp16 (full-rate 1 cycle/row on the PE, fp32 PSUM
accumulation).

v2 restructure vs baseline (310 us):
 - attention emitted as ONE software-pipelined stream of si-steps across
   all 16 (pair, tcj) blocks: scores for step i+2 are emitted before the
   PV matmuls of step i, so the PE never sits on an exp dependency.
 - causal mask-mul narrowed to the 128 diagonal columns ([128,2,128] per
   diag step, single DVE op) instead of two [128,512-f0] muls.
 - PV PSUM banks are freed by fast raw copies; softmax normalization
   (recip + broadcast + mul) runs off the critical path.
 - filler matmul work (qkv projection of the next t-round, output
   projection of the previous round) is emitted in ~2-MM micro-ops
   between attention steps with a supply-balancing quota, with deadline
   flushes to guarantee availability.
 - PSUM budget: 2x scores [128,2,512] (4 banks) + 2 PV accumulators
   (2 banks) + 2 filler (2 banks) = 8 banks.
"""

import math
import os
from collections import deque

import numpy as np

import concourse.bass as bass
import concourse.mybir as mybir
from concourse import bacc
from concourse.tile import TileContext

F32 = mybir.dt.float32
F16 = mybir.dt.float16

N_EMBD = 1024
N_HEAD = 16
D = 64
B = 4
T = 2048
N_CORES = 8
PAIRS = 4          # head pairs per core (8 heads)
TJ = T // 512      # 512-wide t super-chunks
SJ = T // 128      # 128-wide s chunks
SCALE = 1.0 / math.sqrt(D)

_CACHE = {}


def _build():
    nc = bacc.Bacc()

    xt_d = nc.declare_dram_parameter("xt", [N_EMBD, T], F16, isOutput=False)
    wqk_d = nc.declare_dram_parameter("wqk", [N_EMBD, 1024], F16, isOutput=False)
    wv_d = nc.declare_dram_parameter("wv", [N_EMBD, 512], F16, isOutput=False)
    wp_d = nc.declare_dram_parameter("wp", [512, N_EMBD], F16, isOutput=False)
    bqk_d = nc.declare_dram_parameter("bqk", [128, 8], F32, isOutput=False)
    bv_d = nc.declare_dram_parameter("bv", [1, 512], F16, isOutput=False)
    out_d = nc.declare_dram_parameter("out_p", [T, N_EMBD], F32, isOutput=True)

    with TileContext(nc) as tc:
        with (
            tc.tile_pool(name="const", bufs=1) as cpool,
            tc.tile_pool(name="w", bufs=1) as wpool,
            tc.tile_pool(name="xt", bufs=20) as xpool,
            tc.tile_pool(name="qkt", bufs=1) as qkpool,
            tc.tile_pool(name="v", bufs=1) as vpool,
            tc.tile_pool(name="e", bufs=6) as epool,
            tc.tile_pool(name="octp", bufs=1) as octpool,
            tc.tile_pool(name="oraw", bufs=2) as rawpool,
            tc.tile_pool(name="z", bufs=2) as zpool,
            tc.tile_pool(name="rzb", bufs=2) as rzbpool,
            tc.tile_pool(name="outp", bufs=3) as opool,
            tc.tile_pool(name="ps", bufs=2, space="PSUM") as pspool,
            tc.tile_pool(name="pv", bufs=2, space="PSUM") as pvpool,
            tc.tile_pool(name="fp", bufs=2, space="PSUM") as fpool,
        ):
            # ---- constants ----
            ones_f = cpool.tile([1, 128], F32, tag="ones_f")
            nc.vector.memset(ones_f, 1.0)
            ones_r = cpool.tile([1, 128], F16, tag="ones_r")
            nc.vector.tensor_copy(ones_r, ones_f)
            ones8 = cpool.tile([128, 8], F32, tag="ones8")
            nc.vector.memset(ones8, 1.0)
            # warm the ACT exp table while DMAs run
            actw = cpool.tile([1, 8], F32, tag="actw")
            nc.scalar.activation(
                out=actw, in_=ones_f[0:1, 0:8],
                func=mybir.ActivationFunctionType.Exp,
            )
            bqk_t = cpool.tile([128, 8], F32, tag="bqk")
            nc.sync.dma_start(out=bqk_t, in_=bqk_d[:, :])
            bv_t = cpool.tile([1, 512], F16, tag="bv")
            nc.sync.dma_start(out=bv_t, in_=bv_d[:, :])

            # causal mask for the 128 diagonal cols: keep where j - p >= 0
            maskt = cpool.tile([128, 2, 128], F16, tag="maskt")
            nc.vector.memset(maskt, 1.0)
            nc.gpsimd.affine_select(
                out=maskt, in_=maskt, compare_op=mybir.AluOpType.is_ge,
                fill=0.0, base=0, pattern=[[0, 2], [1, 128]],
                channel_multiplier=-1,
            )

            # ---- weights (wv first: the V pass is the first consumer) ----
            wqk = []
            wv = []
            wp = []
            xts = [None] * TJ

            def load_xt(tj):
                lst = []
                for c in range(8):
                    t = xpool.tile([128, 512], F16, tag="xt")
                    nc.sync.dma_start(
                        out=t,
                        in_=xt_d[128 * c : 128 * c + 128, 512 * tj : 512 * tj + 512],
                    )
                    lst.append(t)
                xts[tj] = lst

            xts0 = []
            for c in range(8):
                t = wpool.tile([128, 512], F16, tag=f"wv{c}")
                nc.sync.dma_start(out=t, in_=wv_d[128 * c : 128 * c + 128, :])
                wv.append(t)
                t2_ = xpool.tile([128, 512], F16, tag="xt")
                nc.sync.dma_start(out=t2_, in_=xt_d[128 * c : 128 * c + 128, 0:512])
                xts0.append(t2_)
            xts[0] = xts0

            for c in range(8):
                t = wpool.tile([128, 1024], F16, tag=f"wqk{c}")
                nc.sync.dma_start(out=t, in_=wqk_d[128 * c : 128 * c + 128, :])
                wqk.append(t)
            for p in range(PAIRS):
                t = wpool.tile([128, 1024], F16, tag=f"wp{p}")
                nc.sync.dma_start(out=t, in_=wp_d[128 * p : 128 * p + 128, :])
                wp.append(t)

            # bvb = b_v broadcast to [128, 512] via K=1 matmul
            ps_bvb = fpool.tile([128, 512], F32, tag="fp")
            nc.tensor.matmul(
                ps_bvb, lhsT=ones_r[0:1, :], rhs=bv_t, start=True, stop=True
            )
            bvb = cpool.tile([128, 512], F32, tag="bvb")
            nc.vector.tensor_copy(bvb, ps_bvb)

            QT = [[None] * TJ for _ in range(PAIRS)]
            KT = [[None] * TJ for _ in range(PAIRS)]
            V = [None] * SJ
            OCT = [[None] * TJ for _ in range(PAIRS)]

            # ---------- filler generators (each yield ~= 2 matmuls) ----------
            def gen_v(tj, sj):
                s_idx = 4 * tj + sj
                pvt = fpool.tile([128, 512], F32, tag="fp")
                for c in range(0, 8, 2):
                    for cc in (c, c + 1):
                        nc.tensor.matmul(
                            pvt,
                            lhsT=xts[tj][cc][:, 128 * sj : 128 * sj + 128],
                            rhs=wv[cc],
                            start=(cc == 0),
                            stop=(cc == 7),
                        )
                    if c < 6:
                        yield
                vt = vpool.tile([128, 8, 65], F16, tag=f"v{s_idx}")
                nc.vector.tensor_add(
                    vt[:, :, 0:64],
                    pvt.rearrange("p (h d) -> p h d", h=8),
                    bvb.rearrange("p (h d) -> p h d", h=8),
                )
                nc.vector.tensor_copy(
                    vt[:, :, 64:65], ones8.rearrange("p (h o) -> p h o", h=8)
                )
                V[s_idx] = vt

            def gen_qk(tj, n):
                pst = fpool.tile([128, 512], F32, tag="fp")
                for c in range(0, 8, 2):
                    for cc in (c, c + 1):
                        nc.tensor.matmul(
                            pst,
                            lhsT=wqk[cc][:, 128 * n : 128 * n + 128],
                            rhs=xts[tj][cc],
                            start=(cc == 0),
                            stop=(cc == 7),
                        )
                    if c < 6:
                        yield
                dst = qkpool.tile([128, 512], F16, tag=f"qk{n}_{tj}")
                nc.vector.tensor_scalar_add(dst, pst, bqk_t[:, n : n + 1])
                if n < 4:
                    QT[n][tj] = dst
                else:
                    KT[n - 4][tj] = dst

            def gen_proj(tcj, k, on_act=False):
                t2 = 4 * tcj + k
                for cj in range(2):
                    po = fpool.tile([128, 512], F32, tag="fp")
                    for pair in range(PAIRS):
                        nc.tensor.matmul(
                            po,
                            lhsT=OCT[pair][tcj][:, 128 * k : 128 * k + 128],
                            rhs=wp[pair][:, 512 * cj : 512 * cj + 512],
                            start=(pair == 0),
                            stop=(pair == 3),
                        )
                        if pair == 1:
                            yield
                    ot = opool.tile([128, 512], F32, tag="out")
                    if on_act:
                        nc.scalar.copy(ot, po)
                    else:
                        nc.vector.tensor_copy(ot, po)
                    nc.sync.dma_start(
                        out=out_d[
                            128 * t2 : 128 * t2 + 128, 512 * cj : 512 * cj + 512
                        ],
                        in_=ot,
                    )
                    if cj == 0:
                        yield

            # ---------- filler queue ----------
            # entries: [key, gen, xts_tj_needed, units_left]
            fillq = deque()
            n_left = [0]

            def add_fill(key, gen, need_tj):
                fillq.append([key, gen, need_tj, 4])
                n_left[0] += 4

            def pump(n):
                done = 0
                while done < n and fillq:
                    e = fillq[0]
                    if e[2] is not None and xts[e[2]] is None:
                        return
                    try:
                        next(e[1])
                        e[3] -= 1
                        n_left[0] -= 1
                    except StopIteration:
                        n_left[0] -= e[3]
                        fillq.popleft()
                    done += 1

            def flush(key):
                i = 0
                while i < len(fillq):
                    e = fillq[i]
                    if e[0] <= key:
                        for _ in e[1]:
                            pass
                        n_left[0] -= e[3]
                        del fillq[i]
                    else:
                        i += 1

            for p in range(1, 4):
                add_fill((0, p), gen_qk(0, p), 0)
                add_fill((0, p), gen_qk(0, 4 + p), 0)
            for tj in range(1, TJ):
                for sj in range(4):
                    add_fill((tj, 0), gen_v(tj, sj), tj)
                for p in range(4):
                    add_fill((tj, p), gen_qk(tj, p), tj)
                    add_fill((tj, p), gen_qk(tj, 4 + p), tj)

            # ---------- prologue: qkv needed by block (pair0, tcj0) ----------
            for g in (gen_qk(0, 0), gen_qk(0, 4),
                      gen_v(0, 0), gen_v(0, 1), gen_v(0, 2), gen_v(0, 3)):
                for _ in g:
                    pass

            # ---------- attention stream ----------
            steps = []
            for tcj in range(TJ):
                for pair in range(PAIRS):
                    for si in range(4 * tcj + 4):
                        steps.append((tcj, pair, si))

            ET = {}
            cur_pv = {}

            def front(tcj, pair, si):
                qt = QT[pair][tcj]
                kt = KT[pair][si // 4]
                koff = 128 * (si % 4)
                f0 = max(0, 128 * (si - 4 * tcj))
                ps_t = pspool.tile([128, 1024], F32, tag="ps")
                ps = ps_t.rearrange("p (g f) -> p g f", g=2)
                nc.tensor.matmul(
                    ps[:, 0, f0:512],
                    lhsT=kt[0:64, koff : koff + 128],
                    rhs=qt[0:64, f0:512],
                    start=True, stop=True, tile_position=(0, 0),
                )
                nc.tensor.matmul(
                    ps[:, 1, f0:512],
                    lhsT=kt[64:128, koff : koff + 128],
                    rhs=qt[64:128, f0:512],
                    start=True, stop=True, tile_position=(64, 0),
                )
                et = epool.tile([128, 2, 512], F16, tag="e")
                nc.scalar.activation(
                    out=et[:, :, f0:512],
                    in_=ps[:, :, f0:512],
                    func=mybir.ActivationFunctionType.Exp,
                    scale=SCALE,
                )
                if si >= 4 * tcj:
                    nc.vector.tensor_mul(
                        et[:, :, f0 : f0 + 128], et[:, :, f0 : f0 + 128], maskt
                    )
                ET[(tcj, pair, si)] = (et, f0)

            def back(tcj, pair, si):
                nk = 4 * tcj + 4
                if si == 0:
                    pv1 = pvpool.tile([128, 512], F32, tag="pv")
                    pv2 = pvpool.tile([128, 512], F32, tag="pv")
                    cur_pv[(tcj, pair)] = (pv1, pv2)
                pv1, pv2 = cur_pv[(tcj, pair)]
                et, f0 = ET.pop((tcj, pair, si))
                nc.tensor.matmul(
                    pv1[0:65, f0:512],
                    lhsT=V[si][:, 2 * pair, :],
                    rhs=et[:, 0, f0:512],
                    start=(si == 0), stop=(si == nk - 1),
                )
                nc.tensor.matmul(
                    pv2[0:65, f0:512],
                    lhsT=V[si][:, 2 * pair + 1, :],
                    rhs=et[:, 1, f0:512],
                    start=(si == 0), stop=(si == nk - 1),
                )
                if si == nk - 1:
                    # fast PSUM eviction: raw copies free the banks;
                    # normalization continues off the critical path.
                    oraw1 = rawpool.tile([64, 512], F32, tag="oraw1")
                    oraw2 = rawpool.tile([64, 512], F32, tag="oraw2")
                    rz_a = zpool.tile([1, 512], F32, tag="rz_a")
                    rz_b = zpool.tile([1, 512], F32, tag="rz_b")
                    nc.vector.tensor_copy(oraw1, pv1[0:64, :])
                    nc.vector.tensor_copy(rz_a, pv1[64:65, :])
                    nc.vector.tensor_copy(oraw2, pv2[0:64, :])
                    nc.vector.tensor_copy(rz_b, pv2[64:65, :])
                    nc.vector.reciprocal_approx_fast(out=rz_a, in_=rz_a)
                    nc.vector.reciprocal_approx_fast(out=rz_b, in_=rz_b)
                    rzb1 = rzbpool.tile([64, 512], F32, tag="rzb1")
                    rzb2 = rzbpool.tile([64, 512], F32, tag="rzb2")
                    nc.gpsimd.partition_broadcast(rzb1, rz_a)
                    nc.gpsimd.partition_broadcast(rzb2, rz_b)
                    oct_t = octpool.tile([128, 512], F16, tag=f"oct{pair}_{tcj}")
                    nc.vector.tensor_mul(oct_t[0:64, :], oraw1, rzb1)
                    nc.vector.tensor_mul(oct_t[64:128, :], oraw2, rzb2)
                    OCT[pair][tcj] = oct_t
                    if pair == PAIRS - 1:
                        for k in range(4):
                            add_fill((9, 9), gen_proj(tcj, k, on_act=(tcj == 3)),
                                     None)

            n_steps = len(steps)
            for idx in range(n_steps + 2):
                if idx < n_steps:
                    tcj, pair, si = steps[idx]
                    if si == 0:
                        flush((tcj, pair))
                        if pair == 0 and tcj + 1 < TJ:
                            load_xt(tcj + 1)
                    front(tcj, pair, si)
                    rem_steps = n_steps - idx
                    quota = max(1, min(3, round(n_left[0] / rem_steps) - 1))
                    pump(quota)
                if idx >= 2:
                    back(*steps[idx - 2])
                    pump(1)
            flush((99, 99))

    nc.finalize()
    return nc


def _get_nc():
    if "nc" not in _CACHE:
        _CACHE["nc"] = _build()
    return _CACHE["nc"]


def kernel(x, W_qkv, b_qkv, W_proj, b_proj):
    from concourse.bass_utils import run_bass_kernel_spmd

    x = np.asarray(x, dtype=np.float32)
    W_qkv = np.asarray(W_qkv, dtype=np.float32)
    b_qkv = np.asarray(b_qkv, dtype=np.float32)
    W_proj = np.asarray(W_proj, dtype=np.float32)
    b_proj = np.asarray(b_proj, dtype=np.float32)

    in_maps = []
    for core in range(N_CORES):
        b = core // 2
        h2 = core % 2
        o = 512 * h2
        xt = np.ascontiguousarray(x[b].T).astype(np.float16)
        wq = W_qkv[:, o : o + 512]
        wk = W_qkv[:, 1024 + o : 1024 + o + 512]
        wqk = np.ascontiguousarray(np.concatenate([wq, wk], axis=1)).astype(np.float16)
        wv = np.ascontiguousarray(W_qkv[:, 2048 + o : 2048 + o + 512]).astype(np.float16)
        wp = np.ascontiguousarray(W_proj[o : o + 512, :]).astype(np.float16)
        bq = b_qkv[o : o + 512]
        bk = b_qkv[1024 + o : 1024 + o + 512]
        bqk = np.ascontiguousarray(
            np.concatenate([bq, bk]).reshape(8, 128).T
        )
        bv = np.ascontiguousarray(b_qkv[2048 + o : 2048 + o + 512].reshape(1, 512)).astype(np.float16)
        in_maps.append(
            {"xt": xt, "wqk": wqk, "wv": wv, "wp": wp, "bqk": bqk, "bv": bv}
        )

    nc = _get_nc()
    kwargs = {}
    if os.environ.get("BASS_KERNEL_TRACE"):
        kwargs["trace"] = True
    res = run_bass_kernel_spmd(nc, in_maps, core_ids=list(range(N_CORES)), **kwargs)
    _CACHE["last_results"] = res

    out = np.empty((B, T, N_EMBD), dtype=np.float32)
    for b in range(B):
        out[b] = (
            res.results[2 * b]["out_p"]
            + res.results[2 * b + 1]["out_p"]
            + b_proj[None, :]
        )
    return out


# revision 12
# speedup vs baseline: 1.0269x; 1.0143x over previous
"""Causal self-attention Bass kernel for TRN2, 8 NeuronCores.

Sharding: data-parallel over batch (B=4) x tensor-parallel over head halves
(2 groups of 8 heads) = 8 shards, Megatron-style. Each core computes its
batch's qkv projection for its 8 heads, causal attention, and a partial
output projection (its heads' rows of W_proj). The host sums the two
partials per batch and adds b_proj.

All matmul operands are fp16 (full-rate 1 cycle/row on the PE, fp32 PSUM
accumulation).

v2 restructure vs baseline (310 us):
 - attention emitted as ONE software-pipelined stream of si-steps across
   all 16 (pair, tcj) blocks: scores for step i+2 are emitted before the
   PV matmuls of step i, so the PE never sits on an exp dependency.
 - causal mask-mul narrowed to the 128 diagonal columns ([128,2,128] per
   diag step, single DVE op) instead of two [128,512-f0] muls.
 - PV PSUM banks are freed by fast raw copies; softmax normalization
   (recip + broadcast + mul) runs off the critical path.
 - filler matmul work (qkv projection of the next t-round, output
   projection of the previous round) is emitted in ~2-MM micro-ops
   between attention steps with a supply-balancing quota, with deadline
   flushes to guarantee availability.
 - PSUM budget: 2x scores [128,2,512] (4 banks) + 2 PV accumulators
   (2 banks) + 2 filler (2 banks) = 8 banks.
"""

import math
import os
from collections import deque

import numpy as np

import concourse.bass as bass
import concourse.mybir as mybir
from concourse import bacc
from concourse.tile import TileContext

F32 = mybir.dt.float32
F16 = mybir.dt.float16

N_EMBD = 1024
N_HEAD = 16
D = 64
B = 4
T = 2048
N_CORES = 8
PAIRS = 4          # head pairs per core (8 heads)
TJ = T // 512      # 512-wide t super-chunks
SJ = T // 128      # 128-wide s chunks
SCALE = 1.0 / math.sqrt(D)

_CACHE = {}


def _build():
    nc = bacc.Bacc()

    xt_d = nc.declare_dram_parameter("xt", [N_EMBD, T], F16, isOutput=False)
    wqk_d = nc.declare_dram_parameter("wqk", [N_EMBD, 1024], F16, isOutput=False)
    wv_d = nc.declare_dram_parameter("wv", [N_EMBD, 512], F16, isOutput=False)
    wp_d = nc.declare_dram_parameter("wp", [512, N_EMBD], F16, isOutput=False)
    bqk_d = nc.declare_dram_parameter("bqk", [128, 8], F32, isOutput=False)
    bv_d = nc.declare_dram_parameter("bv", [1, 512], F16, isOutput=False)
    out_d = nc.declare_dram_parameter("out_p", [T, N_EMBD], F32, isOutput=True)

    with TileContext(nc) as tc:
        with (
            tc.tile_pool(name="const", bufs=1) as cpool,
            tc.tile_pool(name="w", bufs=1) as wpool,
            tc.tile_pool(name="xt", bufs=3) as xpool,
            tc.tile_pool(name="xt0", bufs=1) as x0pool,
            tc.tile_pool(name="qkt", bufs=1) as qkpool,
            tc.tile_pool(name="v", bufs=1) as vpool,
            tc.tile_pool(name="e", bufs=8) as epool,
            tc.tile_pool(name="octp", bufs=1) as octpool,
            tc.tile_pool(name="oraw", bufs=2) as rawpool,
            tc.tile_pool(name="z", bufs=2) as zpool,
            tc.tile_pool(name="rzb", bufs=2) as rzbpool,
            tc.tile_pool(name="outp", bufs=3) as opool,
            tc.tile_pool(name="ps", bufs=2, space="PSUM") as pspool,
            tc.tile_pool(name="pv", bufs=2, space="PSUM") as pvpool,
            tc.tile_pool(name="fp", bufs=2, space="PSUM") as fpool,
        ):
            # ---- constants ----
            ones_f = cpool.tile([1, 128], F32, tag="ones_f")
            nc.vector.memset(ones_f, 1.0)
            ones_r = cpool.tile([1, 128], F16, tag="ones_r")
            nc.vector.tensor_copy(ones_r, ones_f)
            ones8 = cpool.tile([128, 8], F32, tag="ones8")
            nc.vector.memset(ones8, 1.0)
            # warm the ACT exp table while DMAs run
            actw = cpool.tile([1, 8], F32, tag="actw")
            nc.scalar.activation(
                out=actw, in_=ones_f[0:1, 0:8],
                func=mybir.ActivationFunctionType.Exp,
            )
            bqk_t = cpool.tile([128, 8], F32, tag="bqk")
            nc.sync.dma_start(out=bqk_t, in_=bqk_d[:, :])
            bv_t = cpool.tile([1, 512], F16, tag="bv")
            nc.sync.dma_start(out=bv_t, in_=bv_d[:, :])

            # causal mask for the 128 diagonal cols: keep where j - p >= 0
            maskt = cpool.tile([128, 2, 128], F16, tag="maskt")
            nc.vector.memset(maskt, 1.0)
            nc.gpsimd.affine_select(
                out=maskt, in_=maskt, compare_op=mybir.AluOpType.is_ge,
                fill=0.0, base=0, pattern=[[0, 2], [1, 128]],
                channel_multiplier=-1,
            )

            # ---- weights, batched DMAs (wv/xt first: V pass is the first
            # consumer; tiny head DMAs so the first matmul starts early) ----
            xts = [None] * TJ

            def load_xt(tj):
                t = xpool.tile([128, 8, 512], F16, tag="xtr")
                nc.sync.dma_start(
                    out=t,
                    in_=xt_d[:, 512 * tj : 512 * tj + 512].rearrange(
                        "(c p) t -> p c t", p=128
                    ),
                )
                xts[tj] = [t[:, c, :] for c in range(8)]

            wv0 = wpool.tile([128, 512], F16, tag="wv0")
            nc.sync.dma_start(out=wv0, in_=wv_d[0:128, :])
            xt00 = x0pool.tile([128, 512], F16, tag="xt00")
            nc.sync.dma_start(out=xt00, in_=xt_d[0:128, 0:512])
            wvr = wpool.tile([128, 7, 512], F16, tag="wvr")
            nc.sync.dma_start(
                out=wvr, in_=wv_d[128:1024, :].rearrange("(c p) n -> p c n", p=128)
            )
            xt0r = x0pool.tile([128, 7, 512], F16, tag="xt0r")
            nc.sync.dma_start(
                out=xt0r,
                in_=xt_d[128:1024, 0:512].rearrange("(c p) t -> p c t", p=128),
            )
            wv = [wv0] + [wvr[:, c, :] for c in range(7)]
            xts[0] = [xt00] + [xt0r[:, c, :] for c in range(7)]

            wqka = wpool.tile([128, 4, 1024], F16, tag="wqka")
            nc.sync.dma_start(
                out=wqka, in_=wqk_d[0:512, :].rearrange("(c p) n -> p c n", p=128)
            )
            wqkb = wpool.tile([128, 4, 1024], F16, tag="wqkb")
            nc.sync.dma_start(
                out=wqkb, in_=wqk_d[512:1024, :].rearrange("(c p) n -> p c n", p=128)
            )
            wqk = [wqka[:, c, :] for c in range(4)] + [wqkb[:, c, :] for c in range(4)]
            wpt = wpool.tile([128, 4, 1024], F16, tag="wpt")
            nc.sync.dma_start(
                out=wpt, in_=wp_d.rearrange("(c p) n -> p c n", p=128)
            )
            wp = [wpt[:, p, :] for p in range(PAIRS)]

            # bvb = b_v broadcast to [128, 512] via K=1 matmul
            ps_bvb = fpool.tile([128, 512], F32, tag="fp")
            nc.tensor.matmul(
                ps_bvb, lhsT=ones_r[0:1, :], rhs=bv_t, start=True, stop=True
            )
            bvb = cpool.tile([128, 512], F32, tag="bvb")
            nc.vector.tensor_copy(bvb, ps_bvb)

            QT = [[None] * TJ for _ in range(PAIRS)]
            KT = [[None] * TJ for _ in range(PAIRS)]
            V = [None] * SJ
            OCT = [[None] * TJ for _ in range(PAIRS)]

            # ---------- filler generators (each yield ~= 2 matmuls) ----------
            def gen_v(tj, sj):
                s_idx = 4 * tj + sj
                pvt = fpool.tile([128, 512], F32, tag="fp")
                for c in range(0, 8, 2):
                    for cc in (c, c + 1):
                        nc.tensor.matmul(
                            pvt,
                            lhsT=xts[tj][cc][:, 128 * sj : 128 * sj + 128],
                            rhs=wv[cc],
                            start=(cc == 0),
                            stop=(cc == 7),
                        )
                    if c < 6:
                        yield
                vt = vpool.tile([128, 8, 65], F16, tag=f"v{s_idx}")
                nc.vector.tensor_add(
                    vt[:, :, 0:64],
                    pvt.rearrange("p (h d) -> p h d", h=8),
                    bvb.rearrange("p (h d) -> p h d", h=8),
                )
                nc.vector.tensor_copy(
                    vt[:, :, 64:65], ones8.rearrange("p (h o) -> p h o", h=8)
                )
                V[s_idx] = vt

            def gen_qk(tj, n):
                pst = fpool.tile([128, 512], F32, tag="fp")
                for c in range(0, 8, 2):
                    for cc in (c, c + 1):
                        nc.tensor.matmul(
                            pst,
                            lhsT=wqk[cc][:, 128 * n : 128 * n + 128],
                            rhs=xts[tj][cc],
                            start=(cc == 0),
                            stop=(cc == 7),
                        )
                    if c < 6:
                        yield
                dst = qkpool.tile([128, 512], F16, tag=f"qk{n}_{tj}")
                nc.vector.tensor_scalar_add(dst, pst, bqk_t[:, n : n + 1])
                if n < 4:
                    QT[n][tj] = dst
                else:
                    KT[n - 4][tj] = dst

            def gen_proj(tcj, k, on_act=False):
                t2 = 4 * tcj + k
                for cj in range(2):
                    po = fpool.tile([128, 512], F32, tag="fp")
                    for pair in range(PAIRS):
                        nc.tensor.matmul(
                            po,
                            lhsT=OCT[pair][tcj][:, 128 * k : 128 * k + 128],
                            rhs=wp[pair][:, 512 * cj : 512 * cj + 512],
                            start=(pair == 0),
                            stop=(pair == 3),
                        )
                        if pair == 1:
                            yield
                    ot = opool.tile([128, 512], F32, tag="out")
                    if on_act:
                        nc.scalar.copy(ot, po)
                    else:
                        nc.vector.tensor_copy(ot, po)
                    nc.sync.dma_start(
                        out=out_d[
                            128 * t2 : 128 * t2 + 128, 512 * cj : 512 * cj + 512
                        ],
                        in_=ot,
                    )
                    if cj == 0:
                        yield

            # ---------- filler queue ----------
            # entries: [key, gen, xts_tj_needed, units_left]
            fillq = deque()
            n_left = [0]
            reserve = [0]

            def add_fill(key, gen, need_tj):
                fillq.append([key, gen, need_tj, 4])
                n_left[0] += 4

            def pump(n):
                done = 0
                while done < n and fillq and n_left[0] > reserve[0]:
                    e = fillq[0]
                    if e[2] is not None and xts[e[2]] is None:
                        return
                    try:
                        next(e[1])
                        e[3] -= 1
                        n_left[0] -= 1
                    except StopIteration:
                        n_left[0] -= e[3]
                        fillq.popleft()
                    done += 1

            def flush(key):
                i = 0
                while i < len(fillq):
                    e = fillq[i]
                    if e[0] <= key:
                        for _ in e[1]:
                            pass
                        n_left[0] -= e[3]
                        del fillq[i]
                    else:
                        i += 1

            for p in range(1, 4):
                add_fill((0, p), gen_qk(0, p), 0)
                add_fill((0, p), gen_qk(0, 4 + p), 0)
            for tj in range(1, TJ):
                for sj in range(4):
                    add_fill((tj, 0), gen_v(tj, sj), tj)
                for p in range(4):
                    add_fill((tj, p), gen_qk(tj, p), tj)
                    add_fill((tj, p), gen_qk(tj, 4 + p), tj)

            # ---------- prologue: qkv needed by block (pair0, tcj0) ----------
            for g in (gen_qk(0, 0), gen_qk(0, 4),
                      gen_v(0, 0), gen_v(0, 1), gen_v(0, 2), gen_v(0, 3)):
                for _ in g:
                    pass

            # ---------- attention stream ----------
            steps = []
            for tcj in range(TJ):
                for pair in range(PAIRS):
                    for si in range(4 * tcj + 4):
                        steps.append((tcj, pair, si))

            ET = {}
            cur_pv = {}

            def front(tcj, pair, si):
                qt = QT[pair][tcj]
                kt = KT[pair][si // 4]
                koff = 128 * (si % 4)
                f0 = max(0, 128 * (si - 4 * tcj))
                ps_t = pspool.tile([128, 1024], F32, tag="ps")
                ps = ps_t.rearrange("p (g f) -> p g f", g=2)
                nc.tensor.matmul(
                    ps[:, 0, f0:512],
                    lhsT=kt[0:64, koff : koff + 128],
                    rhs=qt[0:64, f0:512],
                    start=True, stop=True, tile_position=(0, 0),
                )
                nc.tensor.matmul(
                    ps[:, 1, f0:512],
                    lhsT=kt[64:128, koff : koff + 128],
                    rhs=qt[64:128, f0:512],
                    start=True, stop=True, tile_position=(64, 0),
                )
                et = epool.tile([128, 2, 512], F16, tag="e")
                nc.scalar.activation(
                    out=et[:, :, f0:512],
                    in_=ps[:, :, f0:512],
                    func=mybir.ActivationFunctionType.Exp,
                    scale=SCALE,
                )
                if si >= 4 * tcj:
                    nc.vector.tensor_mul(
                        et[:, :, f0 : f0 + 128], et[:, :, f0 : f0 + 128], maskt
                    )
                ET[(tcj, pair, si)] = (et, f0)

            def back(tcj, pair, si):
                nk = 4 * tcj + 4
                if si == 0:
                    pv1 = pvpool.tile([128, 512], F32, tag="pv")
                    pv2 = pvpool.tile([128, 512], F32, tag="pv")
                    cur_pv[(tcj, pair)] = (pv1, pv2)
                pv1, pv2 = cur_pv[(tcj, pair)]
                et, f0 = ET.pop((tcj, pair, si))
                nc.tensor.matmul(
                    pv1[0:65, f0:512],
                    lhsT=V[si][:, 2 * pair, :],
                    rhs=et[:, 0, f0:512],
                    start=(si == 0), stop=(si == nk - 1),
                )
                nc.tensor.matmul(
                    pv2[0:65, f0:512],
                    lhsT=V[si][:, 2 * pair + 1, :],
                    rhs=et[:, 1, f0:512],
                    start=(si == 0), stop=(si == nk - 1),
                )
                if si == nk - 1:
                    # fast PSUM eviction: raw copies free the banks;
                    # normalization continues off the critical path.
                    oraw1 = rawpool.tile([64, 512], F32, tag="oraw1")
                    oraw2 = rawpool.tile([64, 512], F32, tag="oraw2")
                    rz_a = zpool.tile([1, 512], F32, tag="rz_a")
                    rz_b = zpool.tile([1, 512], F32, tag="rz_b")
                    nc.vector.tensor_copy(oraw1, pv1[0:64, :])
                    nc.vector.tensor_copy(rz_a, pv1[64:65, :])
                    nc.vector.tensor_copy(oraw2, pv2[0:64, :])
                    nc.vector.tensor_copy(rz_b, pv2[64:65, :])
                    nc.vector.reciprocal_approx_fast(out=rz_a, in_=rz_a)
                    nc.vector.reciprocal_approx_fast(out=rz_b, in_=rz_b)
                    rzb1 = rzbpool.tile([64, 512], F32, tag="rzb1")
                    rzb2 = rzbpool.tile([64, 512], F32, tag="rzb2")
                    nc.gpsimd.partition_broadcast(rzb1, rz_a)
                    nc.gpsimd.partition_broadcast(rzb2, rz_b)
                    oct_t = octpool.tile([128, 512], F16, tag=f"oct{pair}_{tcj}")
                    nc.vector.tensor_mul(oct_t[0:64, :], oraw1, rzb1)
                    nc.vector.tensor_mul(oct_t[64:128, :], oraw2, rzb2)
                    OCT[pair][tcj] = oct_t
                    if pair == PAIRS - 1:
                        for k in range(4):
                            add_fill((9, 9), gen_proj(tcj, k, on_act=(tcj == 3)),
                                     None)

            n_steps = len(steps)
            for idx in range(n_steps + 2):
                if idx < n_steps:
                    tcj, pair, si = steps[idx]
                    if si == 0:
                        flush((tcj, pair))
                        if pair == 0 and tcj + 1 < TJ:
                            load_xt(tcj + 1)
                        # hold back filler so the ACT-bound final block still
                        # has PE work to interleave
                        if tcj == TJ - 1:
                            reserve[0] = 20 if pair < PAIRS - 1 else 0
                    front(tcj, pair, si)
                    rem_steps = n_steps - idx
                    quota = max(1, min(3, round(n_left[0] / rem_steps) - 1))
                    pump(quota)
                if idx >= 2:
                    back(*steps[idx - 2])
                    pump(1)
            flush((99, 99))

    nc.finalize()
    return nc


def _get_nc():
    if "nc" not in _CACHE:
        _CACHE["nc"] = _build()
    return _CACHE["nc"]


def kernel(x, W_qkv, b_qkv, W_proj, b_proj):
    from concourse.bass_utils import run_bass_kernel_spmd

    x = np.asarray(x, dtype=np.float32)
    W_qkv = np.asarray(W_qkv, dtype=np.float32)
    b_qkv = np.asarray(b_qkv, dtype=np.float32)
    W_proj = np.asarray(W_proj, dtype=np.float32)
    b_proj = np.asarray(b_proj, dtype=np.float32)

    in_maps = []
    for core in range(N_CORES):
        b = core // 2
        h2 = core % 2
        o = 512 * h2
        xt = np.ascontiguousarray(x[b].T).astype(np.float16)
        wq = W_qkv[:, o : o + 512]
        wk = W_qkv[:, 1024 + o : 1024 + o + 512]
        wqk = np.ascontiguousarray(np.concatenate([wq, wk], axis=1)).astype(np.float16)
        wv = np.ascontiguousarray(W_qkv[:, 2048 + o : 2048 + o + 512]).astype(np.float16)
        wp = np.ascontiguousarray(W_proj[o : o + 512, :]).astype(np.float16)
        bq = b_qkv[o : o + 512]
        bk = b_qkv[1024 + o : 1024 + o + 512]
        bqk = np.ascontiguousarray(
            np.concatenate([bq, bk]).reshape(8, 128).T
        )
        bv = np.ascontiguousarray(b_qkv[2048 + o : 2048 + o + 512].reshape(1, 512)).astype(np.float16)
        in_maps.append(
            {"xt": xt, "wqk": wqk, "wv": wv, "wp": wp, "bqk": bqk, "bv": bv}
        )

    nc = _get_nc()
    kwargs = {}
    if os.environ.get("BASS_KERNEL_TRACE"):
        kwargs["trace"] = True
    res = run_bass_kernel_spmd(nc, in_maps, core_ids=list(range(N_CORES)), **kwargs)
    _CACHE["last_results"] = res

    out = np.empty((B, T, N_EMBD), dtype=np.float32)
    for b in range(B):
        out[b] = (
            res.results[2 * b]["out_p"]
            + res.results[2 * b + 1]["out_p"]
            + b_proj[None, :]
        )
    return out


# revision 16
# speedup vs baseline: 1.0728x; 1.0447x over previous
"""Causal self-attention Bass kernel for TRN2, 8 NeuronCores.

Sharding: data-parallel over batch (B=4) x tensor-parallel over head halves
(2 groups of 8 heads) = 8 shards, Megatron-style. Each core computes its
batch's qkv projection for its 8 heads, causal attention, and a partial
output projection (its heads' rows of W_proj). The host sums the two
partials per batch and adds b_proj.

All matmul operands are fp16 (full-rate 1 cycle/row on the PE, fp32 PSUM
accumulation).

v2 restructure vs baseline (310 us):
 - attention emitted as ONE software-pipelined stream of si-steps across
   all 16 (pair, tcj) blocks: scores for step i+2 are emitted before the
   PV matmuls of step i, so the PE never sits on an exp dependency.
 - causal mask-mul narrowed to the 128 diagonal columns ([128,2,128] per
   diag step, single DVE op) instead of two [128,512-f0] muls.
 - PV PSUM banks are freed by fast raw copies; softmax normalization
   (recip + broadcast + mul) runs off the critical path.
 - filler matmul work (qkv projection of the next t-round, output
   projection of the previous round) is emitted in ~2-MM micro-ops
   between attention steps with a supply-balancing quota, with deadline
   flushes to guarantee availability.
 - PSUM budget: 2x scores [128,2,512] (4 banks) + 2 PV accumulators
   (2 banks) + 2 filler (2 banks) = 8 banks.
"""

import math
import os
from collections import deque

import numpy as np

import concourse.bass as bass
import concourse.mybir as mybir
from concourse import bacc
from concourse.tile import TileContext

F32 = mybir.dt.float32
F16 = mybir.dt.float16

N_EMBD = 1024
N_HEAD = 16
D = 64
B = 4
T = 2048
N_CORES = 8
PAIRS = 4          # head pairs per core (8 heads)
TJ = T // 512      # 512-wide t super-chunks
SJ = T // 128      # 128-wide s chunks
SCALE = 1.0 / math.sqrt(D)

_CACHE = {}


def _build():
    nc = bacc.Bacc()

    xt_d = nc.declare_dram_parameter("xt", [128, TJ, 8, 512], F16, isOutput=False)
    wqk_d = nc.declare_dram_parameter("wqk", [128, 8, 1024], F16, isOutput=False)
    wv_d = nc.declare_dram_parameter("wv", [128, 8, 512], F16, isOutput=False)
    wp_d = nc.declare_dram_parameter("wp", [128, 4, 1024], F16, isOutput=False)
    bqk_d = nc.declare_dram_parameter("bqk", [128, 8], F32, isOutput=False)
    bv_d = nc.declare_dram_parameter("bv", [1, 512], F16, isOutput=False)
    out_d = nc.declare_dram_parameter("out_p", [T, N_EMBD], F32, isOutput=True)

    with TileContext(nc) as tc:
        with (
            tc.tile_pool(name="const", bufs=1) as cpool,
            tc.tile_pool(name="w", bufs=1) as wpool,
            tc.tile_pool(name="xt", bufs=3) as xpool,
            tc.tile_pool(name="xt0", bufs=1) as x0pool,
            tc.tile_pool(name="qkt", bufs=1) as qkpool,
            tc.tile_pool(name="v", bufs=1) as vpool,
            tc.tile_pool(name="e", bufs=8) as epool,
            tc.tile_pool(name="octp", bufs=1) as octpool,
            tc.tile_pool(name="pp", bufs=1) as ppool,
            tc.tile_pool(name="z", bufs=2) as zpool,
            tc.tile_pool(name="rzb", bufs=2) as rzbpool,
            tc.tile_pool(name="outp", bufs=3) as opool,
            tc.tile_pool(name="ps", bufs=2, space="PSUM") as pspool,
            tc.tile_pool(name="pv", bufs=2, space="PSUM") as pvpool,
            tc.tile_pool(name="fp", bufs=2, space="PSUM") as fpool,
        ):
            # ---- constants ----
            ones_f = cpool.tile([1, 128], F32, tag="ones_f")
            nc.vector.memset(ones_f, 1.0)
            ones_r = cpool.tile([1, 128], F16, tag="ones_r")
            nc.vector.tensor_copy(ones_r, ones_f)
            ones8 = cpool.tile([128, 8], F32, tag="ones8")
            nc.vector.memset(ones8, 1.0)
            # warm the ACT exp table while DMAs run
            actw = cpool.tile([1, 8], F32, tag="actw")
            nc.scalar.activation(
                out=actw, in_=ones_f[0:1, 0:8],
                func=mybir.ActivationFunctionType.Exp,
            )
            bqk_t = cpool.tile([128, 8], F32, tag="bqk")
            nc.sync.dma_start(out=bqk_t, in_=bqk_d[:, :])
            bv_t = cpool.tile([1, 512], F16, tag="bv")
            nc.sync.dma_start(out=bv_t, in_=bv_d[:, :])

            # causal mask for the 128 diagonal cols: keep where j - p >= 0
            maskt = cpool.tile([128, 2, 128], F16, tag="maskt")
            nc.vector.memset(maskt, 1.0)
            nc.gpsimd.affine_select(
                out=maskt, in_=maskt, compare_op=mybir.AluOpType.is_ge,
                fill=0.0, base=0, pattern=[[0, 2], [1, 128]],
                channel_multiplier=-1,
            )

            # ---- weights, batched DMAs (wv/xt first: V pass is the first
            # consumer; tiny head DMAs so the first matmul starts early) ----
            xts = [None] * TJ

            def load_xt(tj):
                t = xpool.tile([128, 8, 512], F16, tag="xtr")
                nc.sync.dma_start(out=t, in_=xt_d[:, tj, :, :])
                xts[tj] = [t[:, c, :] for c in range(8)]

            wv0 = wpool.tile([128, 512], F16, tag="wv0")
            nc.sync.dma_start(out=wv0, in_=wv_d[:, 0, :])
            xt00 = x0pool.tile([128, 512], F16, tag="xt00")
            nc.sync.dma_start(out=xt00, in_=xt_d[:, 0, 0, :])
            wvr = wpool.tile([128, 7, 512], F16, tag="wvr")
            nc.sync.dma_start(out=wvr, in_=wv_d[:, 1:8, :])
            xt0r = x0pool.tile([128, 7, 512], F16, tag="xt0r")
            nc.sync.dma_start(out=xt0r, in_=xt_d[:, 0, 1:8, :])
            wv = [wv0] + [wvr[:, c, :] for c in range(7)]
            xts[0] = [xt00] + [xt0r[:, c, :] for c in range(7)]

            wqka = wpool.tile([128, 4, 1024], F16, tag="wqka")
            nc.sync.dma_start(out=wqka, in_=wqk_d[:, 0:4, :])
            wqkb = wpool.tile([128, 4, 1024], F16, tag="wqkb")
            nc.sync.dma_start(out=wqkb, in_=wqk_d[:, 4:8, :])
            wqk = [wqka[:, c, :] for c in range(4)] + [wqkb[:, c, :] for c in range(4)]
            wpt = wpool.tile([128, 4, 1024], F16, tag="wpt")
            nc.sync.dma_start(out=wpt, in_=wp_d[:, :, :])
            wp = [wpt[:, p, :] for p in range(PAIRS)]

            # bvb = b_v broadcast to [128, 512] via K=1 matmul
            ps_bvb = fpool.tile([128, 512], F32, tag="fp")
            nc.tensor.matmul(
                ps_bvb, lhsT=ones_r[0:1, :], rhs=bv_t, start=True, stop=True
            )
            bvb = cpool.tile([128, 512], F32, tag="bvb")
            nc.vector.tensor_copy(bvb, ps_bvb)

            QT = [[None] * TJ for _ in range(PAIRS)]
            KT = [[None] * TJ for _ in range(PAIRS)]
            V = [None] * SJ
            OCT = [[None] * TJ for _ in range(PAIRS)]

            # ---------- filler generators (each yield ~= 2 matmuls) ----------
            def gen_v(tj, sj):
                s_idx = 4 * tj + sj
                pvt = fpool.tile([128, 512], F32, tag="fp")
                for c in range(0, 8, 2):
                    for cc in (c, c + 1):
                        nc.tensor.matmul(
                            pvt,
                            lhsT=xts[tj][cc][:, 128 * sj : 128 * sj + 128],
                            rhs=wv[cc],
                            start=(cc == 0),
                            stop=(cc == 7),
                        )
                    if c < 6:
                        yield
                vt = vpool.tile([128, 8, 65], F16, tag=f"v{s_idx}")
                nc.vector.tensor_add(
                    vt[:, :, 0:64],
                    pvt.rearrange("p (h d) -> p h d", h=8),
                    bvb.rearrange("p (h d) -> p h d", h=8),
                )
                nc.vector.tensor_copy(
                    vt[:, :, 64:65], ones8.rearrange("p (h o) -> p h o", h=8)
                )
                V[s_idx] = vt

            def gen_qk(tj, n):
                pst = fpool.tile([128, 512], F32, tag="fp")
                for c in range(0, 8, 2):
                    for cc in (c, c + 1):
                        nc.tensor.matmul(
                            pst,
                            lhsT=wqk[cc][:, 128 * n : 128 * n + 128],
                            rhs=xts[tj][cc],
                            start=(cc == 0),
                            stop=(cc == 7),
                        )
                    if c < 6:
                        yield
                dst = qkpool.tile([128, 512], F16, tag=f"qk{n}_{tj}")
                nc.vector.tensor_scalar_add(dst, pst, bqk_t[:, n : n + 1])
                if n < 4:
                    QT[n][tj] = dst
                else:
                    KT[n - 4][tj] = dst

            def gen_proj(tcj, k):
                t2 = 4 * tcj + k
                for cj in range(2):
                    po = fpool.tile([128, 512], F32, tag="fp")
                    for pair in range(PAIRS):
                        nc.tensor.matmul(
                            po,
                            lhsT=OCT[pair][tcj][:, 128 * k : 128 * k + 128],
                            rhs=wp[pair][:, 512 * cj : 512 * cj + 512],
                            start=(pair == 0),
                            stop=(pair == 3),
                        )
                        if pair == 1:
                            yield
                    ot = opool.tile([128, 512], F32, tag="out")
                    nc.vector.tensor_copy(ot, po)
                    nc.sync.dma_start(
                        out=out_d[
                            128 * t2 : 128 * t2 + 128, 512 * cj : 512 * cj + 512
                        ],
                        in_=ot,
                    )
                    if cj == 0:
                        yield

            PP = [[None] * 2 for _ in range(4)]

            def gen_projA(k):
                # last round: accumulate pairs 0-2 into SBUF partials while
                # the final attention block is still running
                for cj in range(2):
                    po = fpool.tile([128, 512], F32, tag="fp")
                    for pair in range(3):
                        nc.tensor.matmul(
                            po,
                            lhsT=OCT[pair][TJ - 1][:, 128 * k : 128 * k + 128],
                            rhs=wp[pair][:, 512 * cj : 512 * cj + 512],
                            start=(pair == 0),
                            stop=(pair == 2),
                        )
                        if pair == 1:
                            yield
                    pp = ppool.tile([128, 512], F32, tag=f"pp{k}_{cj}")
                    nc.vector.tensor_copy(pp, po)
                    PP[k][cj] = pp
                    if cj == 0:
                        yield

            # ---------- filler queue ----------
            # entries: [key, gen, xts_tj_needed, units_left]
            fillq = deque()
            n_left = [0]

            def add_fill(key, gen, need_tj):
                fillq.append([key, gen, need_tj, 4])
                n_left[0] += 4

            def due_units(key):
                return sum(e[3] for e in fillq if e[0] <= key)

            def pump(n):
                done = 0
                while done < n and fillq:
                    e = fillq[0]
                    if e[2] is not None and xts[e[2]] is None:
                        return
                    try:
                        next(e[1])
                        e[3] -= 1
                        n_left[0] -= 1
                    except StopIteration:
                        n_left[0] -= e[3]
                        fillq.popleft()
                    done += 1

            def flush(key):
                i = 0
                while i < len(fillq):
                    e = fillq[i]
                    if e[0] <= key:
                        for _ in e[1]:
                            pass
                        n_left[0] -= e[3]
                        del fillq[i]
                    else:
                        i += 1

            for p in range(1, 4):
                add_fill((0, p), gen_qk(0, p), 0)
                add_fill((0, p), gen_qk(0, 4 + p), 0)
            for tj in range(1, TJ):
                for sj in range(4):
                    add_fill((tj, 0), gen_v(tj, sj), tj)
                for p in range(4):
                    add_fill((tj, p), gen_qk(tj, p), tj)
                    add_fill((tj, p), gen_qk(tj, 4 + p), tj)

            # ---------- prologue: qkv needed by block (pair0, tcj0) ----------
            for g in (gen_qk(0, 0), gen_qk(0, 4),
                      gen_v(0, 0), gen_v(0, 1), gen_v(0, 2), gen_v(0, 3)):
                for _ in g:
                    pass

            # ---------- attention stream ----------
            steps = []
            for tcj in range(TJ):
                for pair in range(PAIRS):
                    for si in range(4 * tcj + 4):
                        steps.append((tcj, pair, si))

            ET = {}
            cur_pv = {}

            def front(tcj, pair, si):
                qt = QT[pair][tcj]
                kt = KT[pair][si // 4]
                koff = 128 * (si % 4)
                f0 = max(0, 128 * (si - 4 * tcj))
                ps_t = pspool.tile([128, 1024], F32, tag="ps")
                ps = ps_t.rearrange("p (g f) -> p g f", g=2)
                nc.tensor.matmul(
                    ps[:, 0, f0:512],
                    lhsT=kt[0:64, koff : koff + 128],
                    rhs=qt[0:64, f0:512],
                    start=True, stop=True, tile_position=(0, 0),
                )
                nc.tensor.matmul(
                    ps[:, 1, f0:512],
                    lhsT=kt[64:128, koff : koff + 128],
                    rhs=qt[64:128, f0:512],
                    start=True, stop=True, tile_position=(64, 0),
                )
                et = epool.tile([128, 2, 512], F16, tag="e")
                nc.scalar.activation(
                    out=et[:, :, f0:512],
                    in_=ps[:, :, f0:512],
                    func=mybir.ActivationFunctionType.Exp,
                    scale=SCALE,
                )
                if si >= 4 * tcj:
                    nc.vector.tensor_mul(
                        et[:, :, f0 : f0 + 128], et[:, :, f0 : f0 + 128], maskt
                    )
                ET[(tcj, pair, si)] = (et, f0)

            def back(tcj, pair, si):
                nk = 4 * tcj + 4
                if si == 0:
                    pv1 = pvpool.tile([128, 512], F32, tag="pv")
                    pv2 = pvpool.tile([128, 512], F32, tag="pv")
                    cur_pv[(tcj, pair)] = (pv1, pv2)
                pv1, pv2 = cur_pv[(tcj, pair)]
                et, f0 = ET.pop((tcj, pair, si))
                nc.tensor.matmul(
                    pv1[0:65, f0:512],
                    lhsT=V[si][:, 2 * pair, :],
                    rhs=et[:, 0, f0:512],
                    start=(si == 0), stop=(si == nk - 1),
                )
                nc.tensor.matmul(
                    pv2[0:65, f0:512],
                    lhsT=V[si][:, 2 * pair + 1, :],
                    rhs=et[:, 1, f0:512],
                    start=(si == 0), stop=(si == nk - 1),
                )
                if si == nk - 1:
                    # normalize: recip of the Z rows, broadcast, then scale
                    # the O rows straight out of PSUM.
                    rz_a = zpool.tile([1, 512], F32, tag="rz_a")
                    rz_b = zpool.tile([1, 512], F32, tag="rz_b")
                    nc.vector.tensor_copy(rz_a, pv1[64:65, :])
                    nc.vector.tensor_copy(rz_b, pv2[64:65, :])
                    nc.vector.reciprocal_approx_fast(out=rz_a, in_=rz_a)
                    nc.vector.reciprocal_approx_fast(out=rz_b, in_=rz_b)
                    rzb1 = rzbpool.tile([64, 512], F32, tag="rzb1")
                    rzb2 = rzbpool.tile([64, 512], F32, tag="rzb2")
                    nc.gpsimd.partition_broadcast(rzb1, rz_a)
                    nc.gpsimd.partition_broadcast(rzb2, rz_b)
                    oct_t = octpool.tile([128, 512], F16, tag=f"oct{pair}_{tcj}")
                    nc.vector.tensor_mul(oct_t[0:64, :], pv1[0:64, :], rzb1)
                    nc.vector.tensor_mul(oct_t[64:128, :], pv2[0:64, :], rzb2)
                    OCT[pair][tcj] = oct_t
                    if pair == PAIRS - 1 and tcj < TJ - 1:
                        for k in range(4):
                            add_fill((9, 9), gen_proj(tcj, k), None)
                    if tcj == TJ - 1 and pair == PAIRS - 2:
                        for k in range(4):
                            add_fill((9, 9), gen_projA(k), None)

            n_steps = len(steps)
            steps_left_round = [0] * TJ
            for (tcj, _, _) in steps:
                steps_left_round[tcj] += 1
            for idx in range(n_steps + 2):
                if idx < n_steps:
                    tcj, pair, si = steps[idx]
                    if si == 0:
                        flush((tcj, pair))
                        if pair == 0 and tcj + 1 < TJ:
                            load_xt(tcj + 1)
                    front(tcj, pair, si)
                    due = due_units((tcj + 1, 0))
                    quota = max(1, min(4, -(-due // max(1, steps_left_round[tcj]))))
                    steps_left_round[tcj] -= 1
                    pump(quota)
                if idx >= 2:
                    back(*steps[idx - 2])
            flush((99, 99))

            # tail: pair-3 projection of the last round + partial add
            for k in range(4):
                for cj in range(2):
                    po = fpool.tile([128, 512], F32, tag="fp")
                    nc.tensor.matmul(
                        po,
                        lhsT=OCT[3][TJ - 1][:, 128 * k : 128 * k + 128],
                        rhs=wp[3][:, 512 * cj : 512 * cj + 512],
                        start=True, stop=True,
                    )
                    t2 = 4 * (TJ - 1) + k
                    ot = opool.tile([128, 512], F32, tag="out")
                    nc.vector.tensor_add(ot, po, PP[k][cj])
                    nc.sync.dma_start(
                        out=out_d[
                            128 * t2 : 128 * t2 + 128, 512 * cj : 512 * cj + 512
                        ],
                        in_=ot,
                    )

    nc.finalize()
    return nc


def _get_nc():
    if "nc" not in _CACHE:
        _CACHE["nc"] = _build()
    return _CACHE["nc"]


def kernel(x, W_qkv, b_qkv, W_proj, b_proj):
    from concourse.bass_utils import run_bass_kernel_spmd

    x = np.asarray(x, dtype=np.float32)
    W_qkv = np.asarray(W_qkv, dtype=np.float32)
    b_qkv = np.asarray(b_qkv, dtype=np.float32)
    W_proj = np.asarray(W_proj, dtype=np.float32)
    b_proj = np.asarray(b_proj, dtype=np.float32)

    in_maps = []
    for core in range(N_CORES):
        b = core // 2
        h2 = core % 2
        o = 512 * h2
        # pre-transposed layouts: [partition, chunk, free] with contiguous
        # per-partition runs so each DMA is ~128 descriptors
        xt = x[b].T.astype(np.float16)                       # (1024, 2048)
        xt = np.ascontiguousarray(
            xt.reshape(8, 128, 4, 512).transpose(1, 2, 0, 3)
        )                                                    # (128, 4, 8, 512)
        wq = W_qkv[:, o : o + 512]
        wk = W_qkv[:, 1024 + o : 1024 + o + 512]
        wqk = np.concatenate([wq, wk], axis=1).astype(np.float16)
        wqk = np.ascontiguousarray(wqk.reshape(8, 128, 1024).transpose(1, 0, 2))
        wv = W_qkv[:, 2048 + o : 2048 + o + 512].astype(np.float16)
        wv = np.ascontiguousarray(wv.reshape(8, 128, 512).transpose(1, 0, 2))
        wp = W_proj[o : o + 512, :].astype(np.float16)
        wp = np.ascontiguousarray(wp.reshape(4, 128, 1024).transpose(1, 0, 2))
        bq = b_qkv[o : o + 512]
        bk = b_qkv[1024 + o : 1024 + o + 512]
        bqk = np.ascontiguousarray(
            np.concatenate([bq, bk]).reshape(8, 128).T
        )
        bv = np.ascontiguousarray(b_qkv[2048 + o : 2048 + o + 512].reshape(1, 512)).astype(np.float16)
        in_maps.append(
            {"xt": xt, "wqk": wqk, "wv": wv, "wp": wp, "bqk": bqk, "bv": bv}
        )

    nc = _get_nc()
    kwargs = {}
    if os.environ.get("BASS_KERNEL_TRACE"):
        kwargs["trace"] = True
    res = run_bass_kernel_spmd(nc, in_maps, core_ids=list(range(N_CORES)), **kwargs)
    _CACHE["last_results"] = res

    out = np.empty((B, T, N_EMBD), dtype=np.float32)
    for b in range(B):
        out[b] = (
            res.results[2 * b]["out_p"]
            + res.results[2 * b + 1]["out_p"]
            + b_proj[None, :]
        )
    return out
